# revision 9
# baseline (speedup 1.0000x reference)
"""Multi-head self-attention TRN2 Bass kernel, v2.

Key ideas vs baseline:
- All matmul operands bf16 (1 cyc/row on PE vs ~2 for fp32 modes, and
  keeps the HAM clock-gate warm). fp32 PSUM accumulation throughout.
- Token compaction: ~50% of tokens are masked invalid; invalid keys
  contribute nothing (weights forced to 0) and invalid query rows are
  zeroed. Both are dropped on the host before the kernel runs, so the
  whole attention pipeline (dots/exp/AV) runs on ~half the tokens =
  ~1/4 the work, and projections on ~half.
- Sharding: core c -> (batch = c//2, head-half = c%2). Each core does
  Q/K/V projections for its 512 dims over its batch's compacted
  tokens, attention for its 8 heads, and the row-parallel half of the
  output projection. Host sums core pairs and scatters rows back.
- V is projected directly in [token, dim] layout (stationary = x
  tile), which is exactly the AV-stationary layout: no PE transposes.
  V' = [(V + bv) * inv, inv] per head; the extra inv column makes the
  AV matmul emit the softmax denominator as row 64, with padded keys
  masked for free.
- exp runs as ONE ScalarE activation per (head, ktile) over the whole
  [128 keys x QDEV queries] row (PSUM 3D AP across banks) to amortize
  the ~352-cycle ACTIVATE overhead; ScalarE is the attention-phase
  bottleneck engine.
- Queries beyond 1024 per batch (rare tail, PSUM-bank limit) are
  handled on the host in numpy.
"""

import math
import os
import numpy as np
import ml_dtypes

import concourse.bacc as bacc
import concourse.bass as bass
import concourse.mybir as mybir
from concourse.tile import TileContext
from concourse.bass_utils import run_bass_kernel_spmd

B, S, H, NH, HD = 4, 2048, 1024, 16, 64
NCORES = 8
HPC = 8                   # heads per core
PD = HPC * HD             # per-core projection dim = 512
FT = H // 128             # 8 feature k-tiles
F32 = mybir.dt.float32
BF16 = mybir.dt.bfloat16
NPBF16 = ml_dtypes.bfloat16

LAST_RESULTS = None


def _chunks(total, step=512):
    return [(i * step, min(step, total - i * step))
            for i in range(math.ceil(total / step))]


def build_bass(KTn, QDEV):
    CK = KTn * 128
    ncq = math.ceil(QDEV / 512)
    QP = ncq * 512
    qch = _chunks(QDEV)
    kch = _chunks(CK)

    nc = bacc.Bacc()
    xT = nc.dram_tensor("xT", [H, CK], BF16, kind="ExternalInput")
    wqT = nc.dram_tensor("wqT", [128, FT * PD], BF16, kind="ExternalInput")
    wkT = nc.dram_tensor("wkT", [128, FT * PD], BF16, kind="ExternalInput")
    wvT = nc.dram_tensor("wvT", [128, FT * PD], BF16, kind="ExternalInput")
    wcT = nc.dram_tensor("wcT", [128, 4 * H], BF16, kind="ExternalInput")
    bqk = nc.dram_tensor("bqk", [128, 8], F32, kind="ExternalInput")
    bvb = nc.dram_tensor("bvb", [128, PD], BF16, kind="ExternalInput")
    invc = nc.dram_tensor("invc", [128, KTn], F32, kind="ExternalInput")
    invrep = nc.dram_tensor("invrep", [128, KTn * 8], BF16,
                            kind="ExternalInput")
    eyec = nc.dram_tensor("eyec", [128, 128], BF16, kind="ExternalInput")
    ones = nc.dram_tensor("ones", [1, 64], BF16, kind="ExternalInput")
    outp = nc.dram_tensor("out", [QDEV, H], BF16, kind="ExternalOutput")

    EXP = mybir.ActivationFunctionType.Exp

    with nc.allow_low_precision(reason="bf16 compute validated vs np64"), \
         TileContext(nc) as tc, \
         tc.tile_pool(name="consts", bufs=1) as cpool, \
         tc.tile_pool(name="xt", bufs=FT) as xpool, \
         tc.tile_pool(name="sb", bufs=1) as spool, \
         tc.tile_pool(name="vp", bufs=KTn) as vppool, \
         tc.tile_pool(name="vt", bufs=2) as vtpool, \
         tc.tile_pool(name="pt", bufs=3) as ptpool, \
         tc.tile_pool(name="nrm", bufs=2) as npool, \
         tc.tile_pool(name="outsb", bufs=2) as outpool, \
         tc.tile_pool(name="psum", bufs=2, space="PSUM") as pspool:

        # ---- weights / constants ----
        # DMA order matters for the pipeline lead-in: interleave x tiles
        # with the K/Q/V weight tiles (the first projection matmuls need
        # xt[0]+wk[0], not the whole weight set), and defer Wc (only
        # needed by the output projection at the very end).
        wsb = {name: cpool.tile([128, FT * PD], BF16, name=f"w{name}sb")
               for name in ("q", "k", "v")}
        # Staged DMA: everything the first dots/exp iteration needs
        # (wk, wq, small consts, first 512 x columns) streams in first;
        # the rest of x, then wv and wc, follow behind.
        nc.sync.dma_start(out=wsb["k"][:, :], in_=wkT[:, :])
        nc.sync.dma_start(out=wsb["q"][:, :], in_=wqT[:, :])
        bqk_sb = cpool.tile([128, 8], F32, name="bqksb")
        nc.sync.dma_start(out=bqk_sb[:, :], in_=bqk[:, :])
        invc_sb = cpool.tile([128, KTn], F32, name="invcsb")
        nc.sync.dma_start(out=invc_sb[:, :], in_=invc[:, :])
        eyec_sb = cpool.tile([128, 128], BF16, name="eyecsb")
        nc.sync.dma_start(out=eyec_sb[:, :], in_=eyec[:, :])
        ones_sb = cpool.tile([1, 64], BF16, name="onessb")
        nc.sync.dma_start(out=ones_sb[:, :], in_=ones[:, :])
        bvb_sb = cpool.tile([128, PD], BF16, name="bvbsb")
        nc.sync.dma_start(out=bvb_sb[:, :], in_=bvb[:, :])
        invrep_sb = cpool.tile([128, KTn * 8], BF16, name="invrepsb")
        nc.sync.dma_start(out=invrep_sb[:, :], in_=invrep[:, :])
        xc0 = min(512, CK)
        xt = []
        for ft in range(FT):
            t = xpool.tile([128, CK], BF16, tag="xt", name=f"xt{ft}")
            nc.sync.dma_start(out=t[:, 0:xc0],
                              in_=xT[ft * 128:(ft + 1) * 128, 0:xc0])
            xt.append(t)
        nc.sync.dma_start(out=wsb["v"][:, :], in_=wvT[:, :])
        if xc0 < CK:
            for ft in range(FT):
                nc.sync.dma_start(out=xt[ft][:, xc0:CK],
                                  in_=xT[ft * 128:(ft + 1) * 128, xc0:CK])
        wc_sb = cpool.tile([128, 4 * H], BF16, name="wcsb")
        nc.sync.dma_start(out=wc_sb[:, :], in_=wcT[:, :])

        kT = [spool.tile([128, CK], BF16, tag=f"kT{db}", name=f"kT{db}")
              for db in range(4)]
        qT = [spool.tile([128, QP], BF16, tag=f"qT{db}", name=f"qT{db}")
              for db in range(4)]
        onm = [spool.tile([128, QP], BF16, tag=f"on{db}", name=f"on{db}")
               for db in range(4)]

        def proj(db, w, dst, off, width, bias_col):
            ps = pspool.tile([128, ncq, 512], F32, tag="dp", name=f"pj{w}{db}{off}")
            for ft in range(FT):
                nc.tensor.matmul(
                    ps[:, 0, 0:width],
                    wsb[w][:, ft * PD + db * 128:ft * PD + (db + 1) * 128],
                    xt[ft][:, off:off + width],
                    start=(ft == 0), stop=(ft == FT - 1))
            nc.vector.tensor_scalar_add(dst[:, off:off + width],
                                        ps[:, 0, 0:width],
                                        bqk_sb[:, bias_col:bias_col + 1])

        # K/Q projections for the first dim-block, then V (so attention
        # on heads 0/1 can start early). Projections for dim-blocks 1-3
        # are spread through the attention loop of earlier heads as PE
        # filler: the pure dots/exp/AV stream leaves ~25% PE idle per
        # iteration (ScalarE-paced), which trips the HAM clock-gate back
        # to 1.2 GHz; interleaved projection matmuls keep it at 2.4.
        fills = {}
        for db in range(2, 4):
            # K/Q for dim-block db must complete before head 2*db starts;
            # spread the matmul groups across preceding heads as PE
            # filler inside the ScalarE-paced attention stream.
            hk, hq = 2 * db - 3, 2 * db - 2
            for g, (off, width) in enumerate(kch):
                fills.setdefault((hk, min(2 * g, KTn - 1)), []).append(
                    (db, "k", off, width, 4 + db))
            for g, (off, width) in enumerate(qch):
                fills.setdefault((hq, min(3 * g, KTn - 1)), []).append(
                    (db, "q", off, width, db))
        # Only the first K0/Q0 chunks run up front: the first four
        # attention iterations need nothing else, and every remaining
        # projection chunk streams in as PE filler behind them.
        proj(0, "k", kT[0], kch[0][0], kch[0][1], 4)
        proj(0, "q", qT[0], qch[0][0], qch[0][1], 0)

        vp = [None] * KTn

        def vproj(tt):
            vps = pspool.tile([128, 512], F32, tag="av", name=f"vps{tt}")
            for ft in range(FT):
                nc.tensor.matmul(
                    vps[:, :],
                    xt[ft][:, tt * 128:(tt + 1) * 128],
                    wsb["v"][:, ft * PD:(ft + 1) * PD],
                    start=(ft == 0), stop=(ft == FT - 1))
            vpt = vppool.tile([128, 8, 65], BF16, tag="vp", name=f"vp{tt}")
            vtmp = vtpool.tile([128, PD], F32, tag="vt", name=f"vt{tt}")
            nc.vector.tensor_add(vtmp[:, :], vps[:, :], bvb_sb[:, :])
            nc.vector.tensor_scalar_mul(
                vpt[:, :, 0:64],
                vtmp[:, :].rearrange("p (h d) -> p h d", h=8),
                invc_sb[:, tt:tt + 1])
            nc.vector.tensor_copy(
                vpt[:, :, 64:65],
                invrep_sb[:, tt * 8:(tt + 1) * 8].unsqueeze(2))
            vp[tt] = vpt

        for tt in range(min(2, KTn)):
            vproj(tt)

        # ---- attention ----
        # Processed as (query-half, head-pair, ktile): with 512-wide
        # query chunks, dp/av tiles are one PSUM bank per head, so a
        # head PAIR fits [128,2,512]+[65,2,512] with full double
        # buffering in the 8 banks. The pair's two dots matmuls run
        # concurrently on disjoint 64-row PE groups (row tiling), and a
        # single exp activation covers both heads' scores.
        avt, rct = {}, {}
        n_half = ncq
        sched = [(hf, pr, kt) for pr in range(4)
                 for hf in range(n_half) for kt in range(KTn)]
        dpt = {}

        def emit_dots(i):
            hf, pr, kt = sched[i]
            qoff, qw = qch[hf]
            dp = pspool.tile([128, 2, 512], F32, tag="dp",
                             name=f"dp{hf}_{pr}_{kt}")
            for hs in range(2):
                nc.tensor.matmul(
                    dp[:, hs, 0:qw],
                    kT[pr][hs * 64:(hs + 1) * 64,
                           kt * 128:(kt + 1) * 128],
                    qT[pr][hs * 64:(hs + 1) * 64, qoff:qoff + qw],
                    start=True, stop=True)
            dpt[i] = dp

        def emit_expav(i):
            hf, pr, kt = sched[i]
            qoff, qw = qch[hf]
            if kt == 0:
                avt[(hf, pr)] = pspool.tile([65, 2, 512], F32, tag="av",
                                            name=f"av{hf}_{pr}")
            av = avt[(hf, pr)]
            pt = ptpool.tile([128, 2, 512], BF16, tag="pt",
                             name=f"pt{hf}_{pr}_{kt}")
            nc.scalar.activation(pt[:, :, :], dpt.pop(i)[:, :, :], EXP,
                                 scale=0.125)
            d0 = kt * 128
            if qoff <= d0 < qoff + qw:
                dw = min(128, QDEV - d0)
                off = d0 - qoff
                for hs in range(2):
                    nc.vector.tensor_mul(pt[:, hs, off:off + dw],
                                         pt[:, hs, off:off + dw],
                                         eyec_sb[:, 0:dw])
            for hs in range(2):
                nc.tensor.matmul(
                    av[:, hs, 0:qw],
                    vp[kt][:, 2 * pr + hs, :],
                    pt[:, hs, 0:qw],
                    start=(kt == 0), stop=(kt == KTn - 1))

        def norm_a(hf, pr):
            av = avt[(hf, pr)]
            den = npool.tile([1, 2, 512], F32, tag="den",
                             name=f"den{hf}_{pr}")
            nc.vector.tensor_scalar_max(den[:, :, :], av[64:65, :, :],
                                        1e-30)
            rc = npool.tile([1, 2, 512], F32, tag="rc",
                            name=f"rc{hf}_{pr}")
            nc.vector.reciprocal_approx_fast(rc[:, :, :], den[:, :, :])
            rsb = npool.tile([64, 2, 512], F32, tag="rsb",
                             name=f"rsb{hf}_{pr}")
            nc.gpsimd.partition_broadcast(rsb[:, :, :], rc[:, :, :])
            rct[(hf, pr)] = rsb

        def norm_b(hf, pr):
            qoff, qw = qch[hf]
            av, rsb = avt[(hf, pr)], rct[(hf, pr)]
            for hs in range(2):
                nc.vector.tensor_mul(
                    onm[pr][hs * 64:(hs + 1) * 64, qoff:qoff + qw],
                    av[0:64, hs, 0:qw],
                    rsb[:, hs, 0:qw])

        def op_group(hf, tt):
            # output projection for token-tile tt of query-half hf
            qoff, _ = qch[hf]
            t0 = qoff + tt * 128
            wt = min(128, QDEV - t0)
            op = pspool.tile([128, 2, 512], F32, tag="dp",
                             name=f"op{hf}_{tt}")
            for db in range(4):
                for oc in range(2):
                    nc.tensor.matmul(
                        op[0:wt, oc, :],
                        onm[db][:, t0:t0 + wt],
                        wc_sb[:,
                              db * H + oc * 512:db * H + (oc + 1) * 512],
                        start=(db == 0), stop=(db == 3))
            osb = outpool.tile([128, H], BF16, tag="osb",
                               name=f"osb{hf}_{tt}")
            nc.vector.tensor_copy(
                osb[0:wt, :].rearrange("p (c w) -> p c w", c=2),
                op[0:wt, 0:2, :])
            nc.sync.dma_start(out=outp[t0:t0 + wt, :], in_=osb[0:wt, :])

        fills2 = {}
        h2 = min(1, n_half - 1)

        def add_fill(pr, hf, kt, item):
            fills2.setdefault((min(hf, n_half - 1), pr,
                               max(0, min(kt, KTn - 1))), []).append(item)

        # K0/Q0 remainder: chunk g of kT[0] is first consumed at ktile
        # 4*g, chunk g of qT[0] by query-half g.
        for g, (off, width) in enumerate(kch):
            if g:
                add_fill(0, 0, 2 * (g - 1), (0, "k", off, width, 4))
        for g, (off, width) in enumerate(qch):
            if g:
                add_fill(0, 0, 3 + 2 * (g - 1), (0, "q", off, width, 0))
        # pairs 1-3: spread over the preceding pair's two halves.
        for db in range(1, 4):
            prv = db - 1
            for g, (off, width) in enumerate(kch):
                if db == 1:
                    slot = (prv, h2, 2 * min(g, 3))
                else:
                    slot = [(prv, 0, 4), (prv, 0, 6),
                            (prv, h2, 0)][min(g, 2)]
                add_fill(slot[0], slot[1], slot[2],
                         (db, "k", off, width, 4 + db))
            for g, (off, width) in enumerate(qch):
                if g == 0:
                    slot = (prv, h2, 6 if db == 1 else 4)
                else:
                    slot = (db, 0, 0)
                add_fill(slot[0], slot[1], slot[2],
                         (db, "q", off, width, db))

        NI = len(sched)
        for i in range(NI + 1):
            if i < NI:
                emit_dots(i)
                hf, pr, kt = sched[i]
                if hf == 0 and pr == 0 and kt + 2 < KTn:
                    vproj(kt + 2)
                for fdb, fw, foff, fwidth, fbias in fills2.get(
                        (hf, pr, kt), []):
                    proj(fdb, fw, kT[fdb] if fw == "k" else qT[fdb],
                         foff, fwidth, fbias)
                if (hf, pr) != (0, 0):
                    phf, ppr = ((hf - 1, pr) if hf
                                else (n_half - 1, pr - 1))
                    if kt == min(2, KTn - 1):
                        norm_a(phf, ppr)
                    if kt == min(5, KTn - 1):
                        norm_b(phf, ppr)
                # two of half-0's output tiles hide in the last pair's
                # second-half PE slack; they must follow norm_b(0, 3)
                # (emitted above at kt==5) which writes onm[3] half 0.
                if (n_half == 2 and hf == 1 and pr == 3
                        and kt in (min(6, KTn - 1), KTn - 1)
                        and KTn - 1 > 6):
                    op_group(0, 0 if kt == min(6, KTn - 1) else 1)
            if i >= 1:
                emit_expav(i - 1)
        norm_a(n_half - 1, 3)
        norm_b(n_half - 1, 3)

        # remaining output projection: half-0's last tiles first (they
        # are dependency-free and fill the final norm-chain bubble)
        if n_half == 2:
            done = 2 if KTn - 1 > 6 else 0
            for tt in range(done, 4):
                op_group(0, tt)
        qoffL, qwL = qch[n_half - 1]
        for tt in range(math.ceil(qwL / 128)):
            op_group(n_half - 1, tt)

    nc.finalize()
    return nc


def _np_tail(xc, n, qdev, Wq, bq, Wk, bk, Wv, bv, Wc):
    """Attention rows [qdev:n) of a compacted batch, in numpy fp32."""
    t = n - qdev
    q = xc[qdev:n] @ Wq.T + bq
    k = xc @ Wk.T + bk
    v = xc @ Wv.T + bv
    qh = q.reshape(t, NH, HD).transpose(1, 0, 2)
    kh = k.reshape(n, NH, HD).transpose(1, 0, 2)
    vh = v.reshape(n, NH, HD).transpose(1, 0, 2)
    dots = np.einsum("htd,hnd->htn", qh, kh) / 8.0
    P = np.exp(dots)
    P[:, np.arange(t), qdev + np.arange(t)] = 0.0
    den = np.maximum(P.sum(-1, keepdims=True), 1e-30)
    o = np.einsum("htn,hnd->htd", P / den, vh)
    return o.transpose(1, 0, 2).reshape(t, H) @ Wc.T


def _tile_w(w):
    """[R, C] -> SBUF-tiled [128, (R//128)*C]: row ft*128+p -> [p, ft*C:]."""
    R, C = w.shape
    return np.ascontiguousarray(
        w.reshape(R // 128, 128, C).transpose(1, 0, 2).reshape(128, -1)
    ).astype(NPBF16)


_NC_CACHE = {}


def kernel(encoder_outputs, mask, Wq, bq, Wk, bk, Wv, bv, Wc):
    global LAST_RESULTS
    x = np.asarray(encoder_outputs, dtype=np.float32)
    mask = np.asarray(mask)
    Wq, Wk, Wv, Wc = [np.asarray(w, np.float32) for w in (Wq, Wk, Wv, Wc)]
    bq, bk, bv = [np.asarray(v, np.float32) for v in (bq, bk, bv)]

    validx = [np.where(mask[b] == 0)[0] for b in range(B)]
    nb = [len(v) for v in validx]
    CNT = max(nb)
    out = np.zeros((B, S, H), dtype=np.float32)
    if CNT == 0:
        return out
    KTn = math.ceil(CNT / 128)
    CK = KTn * 128
    QDEV = min(CNT, 1024)

    key = (KTn, QDEV)
    if key not in _NC_CACHE:
        _NC_CACHE[key] = build_bass(KTn, QDEV)
    nc = _NC_CACHE[key]

    eyecm = (1.0 - np.eye(128)).astype(NPBF16)
    onesv = np.ones((1, 64), dtype=NPBF16)
    in_maps = []
    for c in range(NCORES):
        b, hh = c // 2, c % 2
        sl = slice(hh * PD, (hh + 1) * PD)
        xc = x[b][validx[b]]                      # [nb, H]
        xTc = np.zeros((H, CK), dtype=NPBF16)
        xTc[:, :nb[b]] = xc.T
        iv = np.zeros((128, KTn), dtype=np.float32)
        tok = np.arange(CK).reshape(KTn, 128).T   # [128, KTn]
        iv[tok < nb[b]] = 1.0
        in_maps.append({
            "xT": xTc,
            "wqT": _tile_w(Wq[sl, :].T),
            "wkT": _tile_w(Wk[sl, :].T),
            "wvT": _tile_w(Wv[sl, :].T),
            "wcT": _tile_w(Wc[:, sl].T),
            "bqk": np.concatenate(
                [bq[sl].reshape(4, 128).T, bk[sl].reshape(4, 128).T],
                axis=1).copy(),
            "bvb": np.broadcast_to(bv[sl], (128, PD)).astype(NPBF16),
            "invc": iv,
            "invrep": np.repeat(iv, 8, axis=1).astype(NPBF16),
            "eyec": eyecm,
            "ones": onesv,
        })

    res = run_bass_kernel_spmd(
        nc, in_maps, list(range(NCORES)),
        trace=bool(os.environ.get("BASS_TRACE")))
    LAST_RESULTS = res

    for b in range(B):
        if nb[b] == 0:
            continue
        if nb[b] < 8:
            # degenerate batch (kernel drops the denominator zero-guard)
            xc = x[b][validx[b]]
            out[b][validx[b]] = _np_tail(
                xc, nb[b], 0, Wq, bq, Wk, bk, Wv, bv, Wc)
            continue
        ob = (res.results[2 * b]["out"].astype(np.float32) +
             res.results[2 * b + 1]["out"].astype(np.float32))
        rows = min(nb[b], QDEV)
        out[b][validx[b][:rows]] = ob[:rows]
        if nb[b] > QDEV:
            xc = x[b][validx[b]]
            out[b][validx[b][QDEV:]] = _np_tail(
                xc, nb[b], QDEV, Wq, bq, Wk, bk, Wv, bv, Wc)
    return out
